# revision 34
# baseline (speedup 1.0000x reference)
"""Trainium2 Bass kernel for nn_CrossExchangingLayer.

Self-contained: kernel(**inputs) takes full inputs, shards batch-wise over 8
NeuronCores, runs the Bass/Tile kernel, gathers the full output.

Reference computation (B=32, L=512, D=256, H=8, DFF=2048):
  res_i, aw_i = MHA(src_i)  (shared weights, head-averaged attn weights)
  x_i = LN1(src_i + res_i)
  cross-exchange: per-batch median mask on aw_i[:,0,1:] -> replace masked
    tokens of x_i with mean token of the other stream
  out_i = LN2(x_i + FFN(x_i))
Output: np.stack([x1, x2]) of shape [2, B, L, D] float32.
"""
import numpy as np
import ml_dtypes

B, L, D, H, DFF = 32, 512, 256, 8, 2048
HD = D // H          # 32
NC = 8               # cores
BPC = B // NC        # 4 batches per core
EPS = 1e-5
INV_SQRT_HD = 1.0 / np.sqrt(np.float32(HD))

BF16 = ml_dtypes.bfloat16


# ---------------------------------------------------------------------------
# module builder
# ---------------------------------------------------------------------------

def build_module(n_pairs=BPC, debug_seq=None, zero_bias=True, id_affine=True):
    """Build the per-core Bass module processing n_pairs (batch) pairs of
    streams. Sequence s = 2*pair + stream."""
    import concourse.bass as bass
    import concourse.mybir as mybir
    import concourse.tile as tile
    from concourse import bacc
    from concourse.masks import make_identity

    F32 = mybir.dt.float32
    BF = mybir.dt.bfloat16
    U32 = mybir.dt.uint32
    AF = mybir.ActivationFunctionType
    OP = mybir.AluOpType

    nseq = 2 * n_pairs
    nc = bacc.Bacc(None, target_bir_lowering=False, debug=True)

    # ---- DRAM I/O -------------------------------------------------------
    src_h = nc.dram_tensor("src", [nseq, 128, 4, 256], F32, kind="ExternalInput")
    srcT_h = nc.dram_tensor("srcT", [nseq, 128, 2, 512], F32, kind="ExternalInput")
    wqkT_h = nc.dram_tensor("wqkT", [128, 2, 512], BF, kind="ExternalInput")
    wvT_h = nc.dram_tensor("wvT", [128, 2, 256], BF, kind="ExternalInput")
    woutT_h = nc.dram_tensor("woutT", [128, 2, 256], BF, kind="ExternalInput")
    w1T_h = nc.dram_tensor("w1T", [128, 2, 2048], BF, kind="ExternalInput")
    w2T_h = nc.dram_tensor("w2T", [128, 16, 256], BF, kind="ExternalInput")
    wqT32_h = nc.dram_tensor("wqT32", [128, 2, 256], F32, kind="ExternalInput")
    wk32_h = nc.dram_tensor("wk32", [128, 2, 256], F32, kind="ExternalInput")
    bqk_h = nc.dram_tensor("bqk", [128, 4], F32, kind="ExternalInput")
    b1T_h = nc.dram_tensor("b1T", [128, 16], F32, kind="ExternalInput")
    # rows: 0=bv 1=bout 2=b2 3=g1 4=be1 5=g2 6=be2
    rows_h = nc.dram_tensor("rows", [7, 256], F32, kind="ExternalInput")
    out_h = nc.dram_tensor("out", [nseq, 4, 128, 256], F32, kind="ExternalOutput")

    dbg = {}
    if debug_seq is not None:
        for name, shape in [
            ("dbg_qkT", [128, 4, 512]), ("dbg_vaug", [128, 4, 264]),
            ("dbg_PT", [2, 128, 4, 4, 512]), ("dbg_aoT", [128, 2, 512]),
            ("dbg_xpost", [2, 128, 4, 256]), ("dbg_cls", [2, 128, 4]),
            ("dbg_mask", [2, 128, 4]), ("dbg_mean", [2, 128, 256]),
        ]:
            dt = F32 if name != "dbg_vaug" else F32
            dbg[name] = nc.dram_tensor(name, shape, dt, kind="ExternalOutput")

    from contextlib import ExitStack
    with tile.TileContext(nc) as tc, ExitStack() as ctx:
        wp = ctx.enter_context(tc.tile_pool(name="wp", bufs=1))
        sp = ctx.enter_context(tc.tile_pool(name="sp", bufs=2))
        xp = ctx.enter_context(tc.tile_pool(name="xp", bufs=2))
        dp = ctx.enter_context(tc.tile_pool(name="dp", bufs=2, space="DRAM"))
        psS = ctx.enter_context(tc.tile_pool(name="psS", bufs=2, space="PSUM"))
        psU = ctx.enter_context(tc.tile_pool(name="psU", bufs=2, space="PSUM"))
        psM = ctx.enter_context(tc.tile_pool(name="psM", bufs=2, space="PSUM"))

        # ---- constants / weights in SBUF -------------------------------
        wqkT = wp.tile([128, 2, 512], BF)
        wvT = wp.tile([128, 2, 256], BF)
        woutT = wp.tile([128, 2, 256], BF)
        w1T = wp.tile([128, 2, 2048], BF)
        w2T = wp.tile([128, 16, 256], BF)
        wqT32 = wp.tile([128, 2, 256], F32)
        wk32 = wp.tile([128, 2, 256], F32)
        bqk = wp.tile([128, 4], F32)
        b1T = wp.tile([128, 16], F32)
        _weng = [nc.sync, nc.scalar, nc.gpsimd]
        for _i, (t, h) in enumerate([(wqkT, wqkT_h), (wvT, wvT_h), (woutT, woutT_h),
                                     (w1T, w1T_h), (w2T, w2T_h), (wqT32, wqT32_h),
                                     (wk32, wk32_h), (bqk, bqk_h), (b1T, b1T_h)]):
            _weng[_i % 3].dma_start(out=t[:], in_=h[:])
        rowf = []  # bv, bout, b2, g1, be1, g2, be2 materialized [128,256]
        for i, nm in enumerate(["bvf", "boutf", "b2f", "g1f", "be1f", "g2f", "be2f"]):
            need = (not zero_bias) if i < 3 else (not id_affine)
            if need:
                t = wp.tile([128, 256], F32, name=nm)
                nc.sync.dma_start(out=t[:], in_=rows_h[i:i + 1, :].to_broadcast([128, 256]))
            else:
                t = None
            rowf.append(t)
        bvf, boutf, b2f, g1f, be1f, g2f, be2f = rowf
        epst = wp.tile([128, 1], F32)
        nc.vector.memset(epst[:], EPS)
        ones_bf = wp.tile([128, 32], BF)
        nc.vector.memset(ones_bf[:], 1.0)
        onesf = wp.tile([128, 128], F32)
        nc.vector.memset(onesf[:], 1.0)
        idf32 = wp.tile([128, 128], F32)
        make_identity(nc, idf32)


        def act_recip(out, in_):
            eng = nc.scalar
            return eng.add_instruction(mybir.InstActivation(
                name=nc.get_next_instruction_name(),
                func=AF.Reciprocal,
                ins=[eng.lower_ap(in_),
                     mybir.ImmediateValue(dtype=F32, value=0.0),
                     mybir.ImmediateValue(dtype=F32, value=1.0),
                     mybir.ImmediateValue(dtype=F32, value=0.0)],
                outs=[eng.lower_ap(out)],
            ))

        # ---- per-sequence attention + LN1 + mask ------------------------
        def attention(s):
            x_sb = sp.tile([128, 4, 256], F32, tag="x_sb")
            xT = sp.tile([128, 2, 512], F32, tag="xT")
            nc.gpsimd.dma_start(out=x_sb[:], in_=src_h[s])
            nc.sync.dma_start(out=xT[:], in_=srcT_h[s])
            xT_bf = sp.tile([128, 2, 512], BF, tag="xT_bf")
            nc.vector.tensor_copy(out=xT_bf[:], in_=xT[:])

            # qk projection (transposed): qkT[p,mc,t] = qk[t, mc*128+p]
            qkT = sp.tile([128, 4, 512], BF, tag="qkT")
            for mc in range(4):
                pq = psM.tile([128, 512], F32, tag="pm")
                for dc in range(2):
                    nc.tensor.matmul(pq[:], wqkT[:, dc, mc * 128:(mc + 1) * 128],
                                     xT_bf[:, dc, :], start=(dc == 0), stop=(dc == 1))
                if zero_bias:
                    nc.vector.tensor_copy(out=qkT[:, mc, :], in_=pq[:])
                else:
                    nc.vector.tensor_scalar(out=qkT[:, mc, :], in0=pq[:],
                                            scalar1=bqk[:, mc:mc + 1], scalar2=None,
                                            op0=OP.add)

            # v projection (natural) into interleaved v_aug with ones cols
            v_aug = sp.tile([128, 4, 264], BF, tag="v_aug", bufs=3)
            nc.vector.memset(
                v_aug[:].rearrange("p c (h x) -> p c h x", h=8)[:, :, :, 32:33], 1.0)
            for mc in range(4):
                pv = psM.tile([128, 512], F32, tag="pm")
                for dc in range(2):
                    nc.tensor.matmul(pv[:, 0:256], xT_bf[:, dc, mc * 128:(mc + 1) * 128],
                                     wvT[:, dc, :], start=(dc == 0), stop=(dc == 1))
                if zero_bias:
                    nc.vector.tensor_copy(
                        out=v_aug[:, mc, :].rearrange("p (h x) -> p h x", h=8)[:, :, 0:32],
                        in_=pv[:, 0:256].rearrange("p (h x) -> p h x", h=8))
                else:
                    nc.vector.tensor_tensor(
                        out=v_aug[:, mc, :].rearrange("p (h x) -> p h x", h=8)[:, :, 0:32],
                        in0=pv[:, 0:256].rearrange("p (h x) -> p h x", h=8),
                        in1=bvf[:].rearrange("p (h x) -> p h x", h=8),
                        op=OP.add)

            # ---- CLS mask path (f32) -----------------------------------
            # q0 = Wq @ x0 + bq  (transposed, [256] on partitions as [128,2])
            pq0 = psM.tile([128, 512], F32, tag="pm")
            for qc in range(2):
                for dc in range(2):
                    nc.tensor.matmul(pq0[:, qc:qc + 1],
                                     wqT32[:, dc, qc * 128:(qc + 1) * 128],
                                     xT[:, dc, 0:1], start=(dc == 0), stop=(dc == 1))
            q0b = sp.tile([128, 2], F32, tag="q0b")
            if zero_bias:
                nc.vector.tensor_copy(out=q0b[:], in_=pq0[:, 0:2])
            else:
                nc.vector.tensor_tensor(out=q0b[:], in0=pq0[:, 0:2], in1=bqk[:, 0:2], op=OP.add)
            # G[d, h] = sum_i Wk[32h+i, d] * q0[32h+i]
            pG = psM.tile([128, 512], F32, tag="pm")
            for h in range(8):
                hp, hc = 32 * (h % 4), h // 4
                for dcg in range(2):
                    nc.tensor.matmul(pG[:, dcg * 8 + h:dcg * 8 + h + 1],
                                     wk32[hp:hp + 32, hc, dcg * 128:(dcg + 1) * 128],
                                     q0b[hp:hp + 32, hc:hc + 1],
                                     start=True, stop=True, tile_position=(hp, 0))
            G_sb = sp.tile([128, 2, 8], F32, tag="G_sb")
            nc.vector.tensor_copy(out=G_sb[:].rearrange("p a b -> p (a b)"), in_=pG[:, 0:16])
            # s_cls[t, h] = x[t,:] @ G[:, h]
            pC = psM.tile([128, 512], F32, tag="pm")
            for tc in range(4):
                for dc in range(2):
                    nc.tensor.matmul(pC[:, tc * 8:(tc + 1) * 8],
                                     xT[:, dc, tc * 128:(tc + 1) * 128],
                                     G_sb[:, dc, :], start=(dc == 0), stop=(dc == 1))
            E_sb = sp.tile([128, 4, 8], F32, tag="E_sb")
            nc.scalar.activation(out=E_sb[:].rearrange("p a b -> p (a b)"),
                                 in_=pC[:, 0:32], func=AF.Exp, scale=float(INV_SQRT_HD))
            # zc[h] = 8 * sum_t E[t, h]; rz8 = 1/zc broadcast to partitions
            E4 = sp.tile([128, 8], F32, tag="E4")
            nc.vector.reduce_sum(out=E4[:], in_=E_sb[:].rearrange("p a b -> p b a"),
                                 axis=mybir.AxisListType.X)
            pZ = psM.tile([128, 512], F32, tag="pm")
            nc.tensor.matmul(pZ[0:8, 0:1], E4[:], onesf[:, 0:1], start=True, stop=True)
            zc8 = sp.tile([8, 1], F32, tag="zc8")
            nc.vector.tensor_scalar_mul(zc8[:], pZ[0:8, 0:1], 8.0)
            pT2 = psM.tile([128, 512], F32, tag="pm")
            nc.tensor.transpose(pT2[0:1, 0:8], zc8[:], idf32[0:8, 0:8])
            rz8 = sp.tile([1, 8], F32, tag="rz8")
            nc.vector.reciprocal(out=rz8[:], in_=pT2[0:1, 0:8])
            pRz = psM.tile([128, 512], F32, tag="pm")
            nc.tensor.matmul(pRz[:, 0:8], onesf[0:1, :], rz8[:], start=True, stop=True)
            # cls[t] = sum_h E[t,h] * rz[h]
            clsc = sp.tile([128, 4], F32, tag="clsc")
            tmp8 = sp.tile([128, 8], F32, tag="tmp8")
            for tc in range(4):
                nc.vector.tensor_tensor(out=tmp8[:], in0=E_sb[:, tc, :], in1=pRz[:, 0:8],
                                        op=OP.mult)
                nc.vector.reduce_sum(out=clsc[:, tc:tc + 1], in_=tmp8[:],
                                     axis=mybir.AxisListType.X)
            nc.vector.memset(clsc[0:1, 0:1], 3.0e38)  # exclude CLS token
            # rank-count median mask: m[t] = (#{j: cls[j] <= cls[t]} <= 255)
            pCt = psM.tile([128, 512], F32, tag="pm")
            nc.tensor.transpose(pCt[0:4, 0:128], clsc[:], idf32[:, :])
            ct_sb = sp.tile([4, 128], F32, tag="ct_sb")
            nc.vector.tensor_copy(out=ct_sb[:], in_=pCt[0:4, 0:128])
            clsd = dp.tile([4, 128], F32, tag="clsd")
            nc.gpsimd.dma_start(out=clsd[:], in_=ct_sb[:])
            clsfull = sp.tile([128, 512], F32, tag="clsfull")
            nc.gpsimd.dma_start(
                out=clsfull[:],
                in_=clsd[:].rearrange("c p -> (c p)")[None, :].to_broadcast([128, 512]))
            cnt = sp.tile([128, 4], F32, tag="cnt")
            cmpT = sp.tile([128, 512], F32, tag="cmpT")
            for tc in range(4):
                nc.vector.tensor_scalar(out=cmpT[:], in0=clsfull[:],
                                        scalar1=clsc[:, tc:tc + 1], scalar2=None,
                                        op0=OP.is_le, op1=OP.add,
                                        accum_out=cnt[:, tc:tc + 1])
            msk = xp.tile([128, 4], U32, tag=f"msk{s % 2}")
            nc.vector.tensor_scalar(out=msk[:], in0=cnt[:], scalar1=255.0,
                                    scalar2=None, op0=OP.is_le)
            if debug_seq is not None and s in (debug_seq, debug_seq + 1):
                nc.sync.dma_start(out=dbg["dbg_cls"][s % 2], in_=clsc[:])
                mskf = sp.tile([128, 4], F32, tag="mskf")
                nc.vector.tensor_copy(out=mskf[:], in_=msk[:])
                nc.sync.dma_start(out=dbg["dbg_mask"][s % 2], in_=mskf[:])

            # scores (transposed) + exp, 4 heads per group
            PT = sp.tile([128, 4, 4, 512], BF, tag="PT", bufs=3)  # [key, kc, h4, q] per group
            aoT = sp.tile([128, 2, 512], BF, tag="aoT")   # attn-out transposed
            for g in range(2):
                for g2 in range(2):
                    for kc in range(4):
                        pS = psS.tile([128, 2, 512], F32, tag="pS")
                        for hh in range(2):
                            h4 = 2 * g2 + hh
                            nc.tensor.matmul(
                                pS[:, hh, :],
                                qkT[32 * h4:32 * h4 + 32, 2 + g, kc * 128:(kc + 1) * 128],
                                qkT[32 * h4:32 * h4 + 32, g, :],
                                start=True, stop=True, tile_position=(32 * h4, 0))
                        nc.scalar.activation(out=PT[:, kc, 2 * g2:2 * g2 + 2, :],
                                             in_=pS[:], func=AF.Exp,
                                             scale=float(INV_SQRT_HD))
                if debug_seq is not None and s == debug_seq:
                    nc.gpsimd.dma_start(out=dbg["dbg_PT"][g], in_=PT[:])
                # U+Z col-tiled pairs; Z replicated over 32 rows via ones lhsT
                for pr in range(2):
                    ha, hb = 4 * g + 2 * pr, 4 * g + 2 * pr + 1
                    pU = psU.tile([128, 512], F32, tag="pU")
                    for kc in range(4):
                        st, sp_ = (kc == 0), (kc == 3)
                        nc.tensor.matmul(pU[0:32, :], v_aug[:, kc, 33 * ha:33 * ha + 32],
                                         PT[:, kc, 2 * pr, :], start=st, stop=sp_,
                                         tile_position=(0, 0), skip_group_check=True)
                        nc.tensor.matmul(pU[32:64, :], v_aug[:, kc, 33 * hb:33 * hb + 32],
                                         PT[:, kc, 2 * pr + 1, :], start=st, stop=sp_,
                                         tile_position=(0, 32), skip_group_check=True)
                        nc.tensor.matmul(pU[64:96, :], ones_bf[:],
                                         PT[:, kc, 2 * pr, :], start=st, stop=sp_,
                                         tile_position=(0, 64), skip_group_check=True)
                        nc.tensor.matmul(pU[96:128, :], ones_bf[:],
                                         PT[:, kc, 2 * pr + 1, :], start=st, stop=sp_,
                                         tile_position=(0, 96), skip_group_check=True)
                    rz = sp.tile([128, 512], F32, tag="rz")
                    act_recip(rz[64:128, :], pU[64:128, :])
                    nc.vector.tensor_tensor(
                        out=aoT[64 * pr:64 * pr + 64, g, :],
                        in0=pU[0:64, :], in1=rz[64:128, :], op=OP.mult)

            if debug_seq is not None and s == debug_seq:
                nc.gpsimd.dma_start(out=dbg["dbg_qkT"][:], in_=qkT[:])
                dbg_va = sp.tile([128, 4, 264], F32, tag="dbg_va")
                nc.vector.tensor_copy(out=dbg_va[:], in_=v_aug[:])
                nc.sync.dma_start(out=dbg["dbg_vaug"][:], in_=dbg_va[:])
                dbg_ao = sp.tile([128, 2, 512], F32, tag="dbg_ao")
                nc.vector.tensor_copy(out=dbg_ao[:], in_=aoT[:])
                nc.sync.dma_start(out=dbg["dbg_aoT"][:], in_=dbg_ao[:])

            # out-projection + residual + LN1
            xpost = xp.tile([128, 4, 256], F32, tag=f"xpost{s % 2}")
            lnt = sp.tile([128, 4, 256], F32, tag="lnt")
            for mc in range(4):
                pr_ = psM.tile([128, 512], F32, tag="pm")
                for dc in range(2):
                    nc.tensor.matmul(pr_[:, 0:256], aoT[:, dc, mc * 128:(mc + 1) * 128],
                                     woutT[:, dc, :], start=(dc == 0), stop=(dc == 1))
                if not zero_bias:
                    nc.vector.tensor_tensor(out=pr_[:, 0:256], in0=pr_[:, 0:256],
                                            in1=boutf[:], op=OP.add)
                nc.vector.tensor_tensor(out=lnt[:, mc, :], in0=pr_[:, 0:256],
                                        in1=x_sb[:, mc, :], op=OP.add)
            layernorm4(lnt, g1f, be1f, xpost)
            pp = sp.tile([128, 256], F32, tag="pp")
            nc.vector.tensor_tensor(out=pp[:], in0=xpost[:, 0, :], in1=xpost[:, 1, :],
                                    op=OP.add)
            nc.vector.tensor_tensor(out=pp[:], in0=pp[:], in1=xpost[:, 2, :], op=OP.add)
            nc.vector.tensor_tensor(out=pp[:], in0=pp[:], in1=xpost[:, 3, :], op=OP.add)
            pm = psM.tile([128, 512], F32, tag="pm")
            nc.tensor.matmul(pm[:, 0:256], onesf[:], pp[:], start=True, stop=True)
            mf = xp.tile([128, 256], F32, tag=f"mean{s % 2}")
            nc.vector.tensor_scalar_mul(mf[:], pm[:, 0:256], 1.0 / 512.0)

            return xpost, msk, mf

        def layernorm4(t4, gf, bef, out4):
            """LN over last dim of t4 [128,4,256] -> out4 [128,4,256]."""
            mv4 = sp.tile([128, 4, 2], F32, tag="mv4")
            stats4 = sp.tile([128, 4, 6], F32, tag="stats4")
            for mc in range(4):
                nc.vector.bn_stats(out=stats4[:, mc, :], in_=t4[:, mc, :])
                nc.vector.bn_aggr(out=mv4[:, mc, :], in_=stats4[:, mc, :])
            nc.scalar.activation(out=mv4[:, :, 1], in_=mv4[:, :, 1],
                                 func=mybir.ActivationFunctionType.Sqrt, bias=epst[:])
            nc.vector.reciprocal(out=mv4[:, :, 1], in_=mv4[:, :, 1])
            for mc in range(4):
                if id_affine:
                    nc.vector.tensor_scalar(out=out4[:, mc, :], in0=t4[:, mc, :],
                                            scalar1=mv4[:, mc, 0:1], scalar2=mv4[:, mc, 1:2],
                                            op0=mybir.AluOpType.subtract,
                                            op1=mybir.AluOpType.mult)
                else:
                    nc.vector.tensor_scalar(out=t4[:, mc, :], in0=t4[:, mc, :],
                                            scalar1=mv4[:, mc, 0:1], scalar2=mv4[:, mc, 1:2],
                                            op0=mybir.AluOpType.subtract,
                                            op1=mybir.AluOpType.mult)
                    nc.vector.tensor_tensor(out=t4[:, mc, :], in0=t4[:, mc, :],
                                            in1=gf[:], op=mybir.AluOpType.mult)
                    nc.vector.tensor_tensor(out=out4[:, mc, :], in0=t4[:, mc, :],
                                            in1=bef[:], op=mybir.AluOpType.add)

        def ffn_ln2(s, xpost):
            xexT = sp.tile([128, 2, 512], BF, tag="xexT")
            for tc in range(4):
                pt_ = psM.tile([128, 512], F32, tag="pm")
                for dc in range(2):
                    nc.tensor.transpose(pt_[:, dc * 128:(dc + 1) * 128],
                                        xpost[:, tc, dc * 128:(dc + 1) * 128],
                                        idf32[:])
                nc.vector.tensor_copy(
                    out=xexT[:, :, tc * 128:(tc + 1) * 128],
                    in_=pt_[:, 0:256].rearrange("p (dc q) -> p dc q", dc=2))
            hT = sp.tile([128, 16, 512], BF, tag="hT")
            for f in range(16):
                ph = psM.tile([128, 512], F32, tag="pm")
                for dc in range(2):
                    nc.tensor.matmul(ph[:], w1T[:, dc, f * 128:(f + 1) * 128],
                                     xexT[:, dc, :], start=(dc == 0), stop=(dc == 1))
                if zero_bias:
                    if f % 2 == 0:
                        nc.vector.tensor_scalar_max(hT[:, f, :], ph[:], 0.0)
                    else:
                        nc.scalar.activation(out=hT[:, f, :], in_=ph[:],
                                             func=AF.Relu)
                else:
                    nc.vector.tensor_scalar(out=hT[:, f, :], in0=ph[:],
                                            scalar1=b1T[:, f:f + 1], scalar2=0.0,
                                            op0=mybir.AluOpType.add, op1=mybir.AluOpType.max)
            lnt2 = sp.tile([128, 4, 256], F32, tag="lnt2")
            of4 = sp.tile([128, 4, 256], F32, tag="of4")
            for tc in range(4):
                py = psM.tile([128, 512], F32, tag="pm")
                for f in range(16):
                    nc.tensor.matmul(py[:, 0:256], hT[:, f, tc * 128:(tc + 1) * 128],
                                     w2T[:, f, :], start=(f == 0), stop=(f == 15))
                if not zero_bias:
                    nc.vector.tensor_tensor(out=py[:, 0:256], in0=py[:, 0:256],
                                            in1=b2f[:], op=mybir.AluOpType.add)
                nc.vector.tensor_tensor(out=lnt2[:, tc, :], in0=py[:, 0:256],
                                        in1=xpost[:, tc, :], op=mybir.AluOpType.add)
            layernorm4(lnt2, g2f, be2f, of4)
            nc.scalar.dma_start(out=out_h[s].rearrange("c p d -> p c d"), in_=of4[:])

        for pair in range(n_pairs):
            xpostA, mskA, meanA = attention(2 * pair)
            xpostB, mskB, meanB = attention(2 * pair + 1)
            # means of post-LN1 (pre-exchange), replicated on all partitions
            means = [meanA, meanB]
            if debug_seq is not None and 2 * pair == debug_seq:
                nc.sync.dma_start(out=dbg["dbg_mean"][0], in_=means[0][:])
                nc.sync.dma_start(out=dbg["dbg_mean"][1], in_=means[1][:])
            # exchange: replace masked tokens with other stream's mean
            for st, (xpost, msk) in enumerate(((xpostA, mskA), (xpostB, mskB))):
                other = means[1 - st]
                for tc in range(4):
                    nc.vector.copy_predicated(out=xpost[:, tc, :],
                                              mask=msk[:, tc:tc + 1].to_broadcast([128, 256]),
                                              data=other[:])
            if debug_seq is not None and 2 * pair == debug_seq:
                nc.sync.dma_start(out=dbg["dbg_xpost"][0], in_=xpostA[:])
                nc.sync.dma_start(out=dbg["dbg_xpost"][1], in_=xpostB[:])
            ffn_ln2(2 * pair, xpostA)
            ffn_ln2(2 * pair + 1, xpostB)

    nc.compile()
    return nc


# ---------------------------------------------------------------------------
# host packing
# ---------------------------------------------------------------------------

def pack_weights(W_in, b_in, W_out, b_out, W1, b1, W2, b2, g1, be1, g2, be2):
    def chunkp(a, c):  # [c*128, F] -> [128, c, F]
        return np.ascontiguousarray(a.reshape(c, 128, -1).transpose(1, 0, 2))

    wqk = W_in[:2 * D]                     # [512, 256]
    wv = W_in[2 * D:]                      # [256, 256]
    m = {
        "wqkT": chunkp(wqk.T, 2).astype(BF16),      # [128,2,512]
        "wvT": chunkp(wv.T, 2).astype(BF16),        # [128,2,256]
        "woutT": chunkp(W_out.T, 2).astype(BF16),   # [128,2,256]
        "w1T": chunkp(W1.T, 2).astype(BF16),        # [128,2,2048]
        "w2T": chunkp(W2.T, 16).astype(BF16),       # [128,16,256]
        "wqT32": chunkp(W_in[:D].T, 2).astype(np.float32),
        "wk32": chunkp(W_in[D:2 * D], 2).astype(np.float32),
        "bqk": np.ascontiguousarray(b_in[:2 * D].reshape(4, 128).T).astype(np.float32),
        "b1T": np.ascontiguousarray(b1.reshape(16, 128).T).astype(np.float32),
        "rows": np.stack([b_in[2 * D:], b_out, b2, g1, be1, g2, be2]).astype(np.float32),
    }
    return m


def pack_seqs(seqs):
    """seqs: [n, 512, 256] -> src [n,128,4,256], srcT [n,128,2,512]."""
    n = seqs.shape[0]
    src = np.ascontiguousarray(
        seqs.reshape(n, 4, 128, 256).transpose(0, 2, 1, 3)).astype(np.float32)
    st = seqs.transpose(0, 2, 1)  # [n, 256, 512]
    srcT = np.ascontiguousarray(
        st.reshape(n, 2, 128, 512).transpose(0, 2, 1, 3)).astype(np.float32)
    return src, srcT


_MODULE_CACHE = {}


def get_module(n_pairs=BPC, debug_seq=None, zero_bias=True, id_affine=True):
    key = (n_pairs, debug_seq, zero_bias, id_affine)
    if key not in _MODULE_CACHE:
        _MODULE_CACHE[key] = build_module(n_pairs, debug_seq, zero_bias, id_affine)
    return _MODULE_CACHE[key]


def kernel(src1, src2, AT_attn, AV_attn, W_in, b_in, W_out, b_out,
           W1, b1, W2, b2, g1, be1, g2, be2):
    from concourse.bass_utils import run_bass_kernel_spmd

    zb = all(float(np.abs(np.asarray(a)).max()) == 0.0
             for a in (b_in, b_out, b1, b2))
    ia = (float(np.abs(np.asarray(g1) - 1).max()) == 0 and float(np.abs(np.asarray(g2) - 1).max()) == 0
          and float(np.abs(np.asarray(be1)).max()) == 0 and float(np.abs(np.asarray(be2)).max()) == 0)
    nc = get_module(zero_bias=zb, id_affine=ia)
    wm = pack_weights(np.asarray(W_in, np.float32), np.asarray(b_in, np.float32),
                      np.asarray(W_out, np.float32), np.asarray(b_out, np.float32),
                      np.asarray(W1, np.float32), np.asarray(b1, np.float32),
                      np.asarray(W2, np.float32), np.asarray(b2, np.float32),
                      np.asarray(g1, np.float32), np.asarray(be1, np.float32),
                      np.asarray(g2, np.float32), np.asarray(be2, np.float32))
    src1 = np.asarray(src1, np.float32)
    src2 = np.asarray(src2, np.float32)
    in_maps = []
    for c in range(NC):
        bs = slice(c * BPC, (c + 1) * BPC)
        seqs = np.empty((2 * BPC, L, D), np.float32)
        seqs[0::2] = src1[bs]
        seqs[1::2] = src2[bs]
        src, srcT = pack_seqs(seqs)
        in_maps.append({"src": src, "srcT": srcT, **wm})
    res = run_bass_kernel_spmd(nc, in_maps, core_ids=list(range(NC)))
    x1 = np.empty((B, L, D), np.float32)
    x2 = np.empty((B, L, D), np.float32)
    for c in range(NC):
        o = res.results[c]["out"]  # [8, 4, 128, 256]
        full = o.reshape(2 * BPC, 512, 256)
        x1[c * BPC:(c + 1) * BPC] = full[0::2]
        x2[c * BPC:(c + 1) * BPC] = full[1::2]
    return np.stack([x1, x2])


# revision 35
# speedup vs baseline: 1.0349x; 1.0349x over previous
"""Trainium2 Bass kernel for nn_CrossExchangingLayer.

Self-contained: kernel(**inputs) takes full inputs, shards batch-wise over 8
NeuronCores, runs the Bass/Tile kernel, gathers the full output.

Reference computation (B=32, L=512, D=256, H=8, DFF=2048):
  res_i, aw_i = MHA(src_i)  (shared weights, head-averaged attn weights)
  x_i = LN1(src_i + res_i)
  cross-exchange: per-batch median mask on aw_i[:,0,1:] -> replace masked
    tokens of x_i with mean token of the other stream
  out_i = LN2(x_i + FFN(x_i))
Output: np.stack([x1, x2]) of shape [2, B, L, D] float32.
"""
import numpy as np
import ml_dtypes

B, L, D, H, DFF = 32, 512, 256, 8, 2048
HD = D // H          # 32
NC = 8               # cores
BPC = B // NC        # 4 batches per core
EPS = 1e-5
INV_SQRT_HD = 1.0 / np.sqrt(np.float32(HD))

BF16 = ml_dtypes.bfloat16


# ---------------------------------------------------------------------------
# module builder
# ---------------------------------------------------------------------------

def build_module(n_pairs=BPC, debug_seq=None, zero_bias=True, id_affine=True):
    """Build the per-core Bass module processing n_pairs (batch) pairs of
    streams. Sequence s = 2*pair + stream."""
    import concourse.bass as bass
    import concourse.mybir as mybir
    import concourse.tile as tile
    from concourse import bacc
    from concourse.masks import make_identity

    F32 = mybir.dt.float32
    BF = mybir.dt.bfloat16
    U32 = mybir.dt.uint32
    AF = mybir.ActivationFunctionType
    OP = mybir.AluOpType

    nseq = 2 * n_pairs
    nc = bacc.Bacc(None, target_bir_lowering=False, debug=True)

    # ---- DRAM I/O -------------------------------------------------------
    src_h = nc.dram_tensor("src", [nseq, 128, 4, 256], F32, kind="ExternalInput")
    srcT_h = nc.dram_tensor("srcT", [nseq, 128, 2, 512], F32, kind="ExternalInput")
    wqkT_h = nc.dram_tensor("wqkT", [128, 2, 512], BF, kind="ExternalInput")
    wvT_h = nc.dram_tensor("wvT", [128, 2, 256], BF, kind="ExternalInput")
    woutT_h = nc.dram_tensor("woutT", [128, 2, 256], BF, kind="ExternalInput")
    w1T_h = nc.dram_tensor("w1T", [128, 2, 2048], BF, kind="ExternalInput")
    w2T_h = nc.dram_tensor("w2T", [128, 16, 256], BF, kind="ExternalInput")
    wqT32_h = nc.dram_tensor("wqT32", [128, 2, 256], F32, kind="ExternalInput")
    wk32_h = nc.dram_tensor("wk32", [128, 2, 256], F32, kind="ExternalInput")
    bqk_h = nc.dram_tensor("bqk", [128, 4], F32, kind="ExternalInput")
    b1T_h = nc.dram_tensor("b1T", [128, 16], F32, kind="ExternalInput")
    # rows: 0=bv 1=bout 2=b2 3=g1 4=be1 5=g2 6=be2
    rows_h = nc.dram_tensor("rows", [7, 256], F32, kind="ExternalInput")
    out_h = nc.dram_tensor("out", [nseq, 4, 128, 256], F32, kind="ExternalOutput")

    dbg = {}
    if debug_seq is not None:
        for name, shape in [
            ("dbg_qkT", [128, 4, 512]), ("dbg_vaug", [128, 4, 264]),
            ("dbg_PT", [2, 128, 4, 4, 512]), ("dbg_aoT", [128, 2, 512]),
            ("dbg_xpost", [2, 128, 4, 256]), ("dbg_cls", [2, 128, 4]),
            ("dbg_mask", [2, 128, 4]), ("dbg_mean", [2, 128, 256]),
        ]:
            dt = F32 if name != "dbg_vaug" else F32
            dbg[name] = nc.dram_tensor(name, shape, dt, kind="ExternalOutput")

    from contextlib import ExitStack
    with tile.TileContext(nc) as tc, ExitStack() as ctx:
        wp = ctx.enter_context(tc.tile_pool(name="wp", bufs=1))
        sp = ctx.enter_context(tc.tile_pool(name="sp", bufs=2))
        xp = ctx.enter_context(tc.tile_pool(name="xp", bufs=2))
        dp = ctx.enter_context(tc.tile_pool(name="dp", bufs=2, space="DRAM"))
        psS = ctx.enter_context(tc.tile_pool(name="psS", bufs=2, space="PSUM"))
        psU = ctx.enter_context(tc.tile_pool(name="psU", bufs=2, space="PSUM"))
        psM = ctx.enter_context(tc.tile_pool(name="psM", bufs=2, space="PSUM"))

        # ---- constants / weights in SBUF -------------------------------
        wqkT = wp.tile([128, 2, 512], BF)
        wvT = wp.tile([128, 2, 256], BF)
        woutT = wp.tile([128, 2, 256], BF)
        w1T = wp.tile([128, 2, 2048], BF)
        w2T = wp.tile([128, 16, 256], BF)
        wqT32 = wp.tile([128, 2, 256], F32)
        wk32 = wp.tile([128, 2, 256], F32)
        bqk = wp.tile([128, 4], F32)
        b1T = wp.tile([128, 16], F32)
        _weng = [nc.sync, nc.scalar, nc.gpsimd]
        for _i, (t, h) in enumerate([(wqkT, wqkT_h), (wvT, wvT_h), (woutT, woutT_h),
                                     (w1T, w1T_h), (w2T, w2T_h), (wqT32, wqT32_h),
                                     (wk32, wk32_h), (bqk, bqk_h), (b1T, b1T_h)]):
            _weng[_i % 3].dma_start(out=t[:], in_=h[:])
        rowf = []  # bv, bout, b2, g1, be1, g2, be2 materialized [128,256]
        for i, nm in enumerate(["bvf", "boutf", "b2f", "g1f", "be1f", "g2f", "be2f"]):
            need = (not zero_bias) if i < 3 else (not id_affine)
            if need:
                t = wp.tile([128, 256], F32, name=nm)
                nc.sync.dma_start(out=t[:], in_=rows_h[i:i + 1, :].to_broadcast([128, 256]))
            else:
                t = None
            rowf.append(t)
        bvf, boutf, b2f, g1f, be1f, g2f, be2f = rowf
        epst = wp.tile([128, 1], F32)
        nc.vector.memset(epst[:], EPS)
        ones_bf = wp.tile([128, 32], BF)
        nc.vector.memset(ones_bf[:], 1.0)
        onesf = wp.tile([128, 128], F32)
        nc.vector.memset(onesf[:], 1.0)
        idf32 = wp.tile([128, 128], F32)
        make_identity(nc, idf32)


        def act_recip(out, in_):
            eng = nc.scalar
            return eng.add_instruction(mybir.InstActivation(
                name=nc.get_next_instruction_name(),
                func=AF.Reciprocal,
                ins=[eng.lower_ap(in_),
                     mybir.ImmediateValue(dtype=F32, value=0.0),
                     mybir.ImmediateValue(dtype=F32, value=1.0),
                     mybir.ImmediateValue(dtype=F32, value=0.0)],
                outs=[eng.lower_ap(out)],
            ))

        # ---- per-sequence attention + LN1 + mask ------------------------
        def attention(s):
            x_sb = sp.tile([128, 4, 256], F32, tag="x_sb")
            xT = sp.tile([128, 2, 512], F32, tag="xT")
            nc.gpsimd.dma_start(out=x_sb[:], in_=src_h[s])
            nc.sync.dma_start(out=xT[:], in_=srcT_h[s])
            xT_bf = sp.tile([128, 2, 512], BF, tag="xT_bf")
            nc.vector.tensor_copy(out=xT_bf[:], in_=xT[:])

            # qk projection (transposed): qkT[p,mc,t] = qk[t, mc*128+p]
            qkT = sp.tile([128, 4, 512], BF, tag="qkT")
            for mc in range(4):
                pq = psM.tile([128, 512], F32, tag="pm")
                for dc in range(2):
                    nc.tensor.matmul(pq[:], wqkT[:, dc, mc * 128:(mc + 1) * 128],
                                     xT_bf[:, dc, :], start=(dc == 0), stop=(dc == 1))
                if zero_bias:
                    nc.vector.tensor_copy(out=qkT[:, mc, :], in_=pq[:])
                else:
                    nc.vector.tensor_scalar(out=qkT[:, mc, :], in0=pq[:],
                                            scalar1=bqk[:, mc:mc + 1], scalar2=None,
                                            op0=OP.add)

            # v projection (natural) into interleaved v_aug with ones cols
            v_aug = sp.tile([128, 4, 264], BF, tag="v_aug", bufs=3)
            nc.vector.memset(
                v_aug[:].rearrange("p c (h x) -> p c h x", h=8)[:, :, :, 32:33], 1.0)
            for mc in range(4):
                pv = psM.tile([128, 512], F32, tag="pm")
                for dc in range(2):
                    nc.tensor.matmul(pv[:, 0:256], xT_bf[:, dc, mc * 128:(mc + 1) * 128],
                                     wvT[:, dc, :], start=(dc == 0), stop=(dc == 1))
                if zero_bias:
                    nc.vector.tensor_copy(
                        out=v_aug[:, mc, :].rearrange("p (h x) -> p h x", h=8)[:, :, 0:32],
                        in_=pv[:, 0:256].rearrange("p (h x) -> p h x", h=8))
                else:
                    nc.vector.tensor_tensor(
                        out=v_aug[:, mc, :].rearrange("p (h x) -> p h x", h=8)[:, :, 0:32],
                        in0=pv[:, 0:256].rearrange("p (h x) -> p h x", h=8),
                        in1=bvf[:].rearrange("p (h x) -> p h x", h=8),
                        op=OP.add)

            # ---- CLS mask path (f32) -----------------------------------
            # q0 = Wq @ x0 + bq  (transposed, [256] on partitions as [128,2])
            pq0 = psM.tile([128, 512], F32, tag="pm")
            for qc in range(2):
                for dc in range(2):
                    nc.tensor.matmul(pq0[:, qc:qc + 1],
                                     wqT32[:, dc, qc * 128:(qc + 1) * 128],
                                     xT[:, dc, 0:1], start=(dc == 0), stop=(dc == 1))
            q0b = sp.tile([128, 2], F32, tag="q0b")
            if zero_bias:
                nc.vector.tensor_copy(out=q0b[:], in_=pq0[:, 0:2])
            else:
                nc.vector.tensor_tensor(out=q0b[:], in0=pq0[:, 0:2], in1=bqk[:, 0:2], op=OP.add)
            # G[d, h] = sum_i Wk[32h+i, d] * q0[32h+i]
            pG = psM.tile([128, 512], F32, tag="pm")
            for h in range(8):
                hp, hc = 32 * (h % 4), h // 4
                for dcg in range(2):
                    nc.tensor.matmul(pG[:, dcg * 8 + h:dcg * 8 + h + 1],
                                     wk32[hp:hp + 32, hc, dcg * 128:(dcg + 1) * 128],
                                     q0b[hp:hp + 32, hc:hc + 1],
                                     start=True, stop=True, tile_position=(hp, 0))
            G_sb = sp.tile([128, 2, 8], F32, tag="G_sb")
            nc.vector.tensor_copy(out=G_sb[:].rearrange("p a b -> p (a b)"), in_=pG[:, 0:16])
            # s_cls[t, h] = x[t,:] @ G[:, h]
            pC = psM.tile([128, 512], F32, tag="pm")
            for tc in range(4):
                for dc in range(2):
                    nc.tensor.matmul(pC[:, tc * 8:(tc + 1) * 8],
                                     xT[:, dc, tc * 128:(tc + 1) * 128],
                                     G_sb[:, dc, :], start=(dc == 0), stop=(dc == 1))
            E_sb = sp.tile([128, 4, 8], F32, tag="E_sb")
            nc.scalar.activation(out=E_sb[:].rearrange("p a b -> p (a b)"),
                                 in_=pC[:, 0:32], func=AF.Exp, scale=float(INV_SQRT_HD))
            # zc[h] = 8 * sum_t E[t, h]; rz8 = 1/zc broadcast to partitions
            E4 = sp.tile([128, 8], F32, tag="E4")
            nc.vector.reduce_sum(out=E4[:], in_=E_sb[:].rearrange("p a b -> p b a"),
                                 axis=mybir.AxisListType.X)
            pZ = psM.tile([128, 512], F32, tag="pm")
            nc.tensor.matmul(pZ[0:8, 0:1], E4[:], onesf[:, 0:1], start=True, stop=True)
            zc8 = sp.tile([8, 1], F32, tag="zc8")
            nc.vector.tensor_scalar_mul(zc8[:], pZ[0:8, 0:1], 8.0)
            pT2 = psM.tile([128, 512], F32, tag="pm")
            nc.tensor.transpose(pT2[0:1, 0:8], zc8[:], idf32[0:8, 0:8])
            rz8 = sp.tile([1, 8], F32, tag="rz8")
            nc.vector.reciprocal(out=rz8[:], in_=pT2[0:1, 0:8])
            pRz = psM.tile([128, 512], F32, tag="pm")
            nc.tensor.matmul(pRz[:, 0:8], onesf[0:1, :], rz8[:], start=True, stop=True)
            # cls[t] = sum_h E[t,h] * rz[h]
            clsc = sp.tile([128, 4], F32, tag="clsc")
            tmp8 = sp.tile([128, 8], F32, tag="tmp8")
            for tc in range(4):
                nc.vector.tensor_tensor(out=tmp8[:], in0=E_sb[:, tc, :], in1=pRz[:, 0:8],
                                        op=OP.mult)
                nc.vector.reduce_sum(out=clsc[:, tc:tc + 1], in_=tmp8[:],
                                     axis=mybir.AxisListType.X)
            nc.vector.memset(clsc[0:1, 0:1], 3.0e38)  # exclude CLS token
            # rank-count median mask: m[t] = (#{j: cls[j] <= cls[t]} <= 255)
            pCt = psM.tile([128, 512], F32, tag="pm")
            nc.tensor.transpose(pCt[0:4, 0:128], clsc[:], idf32[:, :])
            ct_sb = sp.tile([4, 128], F32, tag="ct_sb")
            nc.vector.tensor_copy(out=ct_sb[:], in_=pCt[0:4, 0:128])
            clsd = dp.tile([4, 128], F32, tag="clsd")
            nc.gpsimd.dma_start(out=clsd[:], in_=ct_sb[:])
            clsfull = sp.tile([128, 512], F32, tag="clsfull")
            nc.gpsimd.dma_start(
                out=clsfull[:],
                in_=clsd[:].rearrange("c p -> (c p)")[None, :].to_broadcast([128, 512]))
            cnt = sp.tile([128, 4], F32, tag="cnt")
            cmpT = sp.tile([128, 512], F32, tag="cmpT")
            for tc in range(4):
                nc.vector.tensor_scalar(out=cmpT[:], in0=clsfull[:],
                                        scalar1=clsc[:, tc:tc + 1], scalar2=None,
                                        op0=OP.is_le, op1=OP.add,
                                        accum_out=cnt[:, tc:tc + 1])
            msk = xp.tile([128, 4], U32, tag=f"msk{s % 2}")
            nc.vector.tensor_scalar(out=msk[:], in0=cnt[:], scalar1=255.0,
                                    scalar2=None, op0=OP.is_le)
            if debug_seq is not None and s in (debug_seq, debug_seq + 1):
                nc.sync.dma_start(out=dbg["dbg_cls"][s % 2], in_=clsc[:])
                mskf = sp.tile([128, 4], F32, tag="mskf")
                nc.vector.tensor_copy(out=mskf[:], in_=msk[:])
                nc.sync.dma_start(out=dbg["dbg_mask"][s % 2], in_=mskf[:])

            # scores (transposed) + exp, 4 heads per group
            PT = sp.tile([128, 4, 4, 512], BF, tag="PT", bufs=3)  # [key, kc, h4, q] per group
            aoT = sp.tile([128, 2, 512], BF, tag="aoT")   # attn-out transposed
            for g in range(2):
                for g2 in range(2):
                    for kc in range(4):
                        pS = psS.tile([128, 2, 512], F32, tag="pS")
                        for hh in range(2):
                            h4 = 2 * g2 + hh
                            nc.tensor.matmul(
                                pS[:, hh, :],
                                qkT[32 * h4:32 * h4 + 32, 2 + g, kc * 128:(kc + 1) * 128],
                                qkT[32 * h4:32 * h4 + 32, g, :],
                                start=True, stop=True, tile_position=(32 * h4, 0))
                        nc.scalar.activation(out=PT[:, kc, 2 * g2:2 * g2 + 2, :],
                                             in_=pS[:], func=AF.Exp,
                                             scale=float(INV_SQRT_HD))
                if debug_seq is not None and s == debug_seq:
                    nc.gpsimd.dma_start(out=dbg["dbg_PT"][g], in_=PT[:])
                # U+Z col-tiled pairs; Z replicated over 32 rows via ones lhsT
                for pr in range(2):
                    ha, hb = 4 * g + 2 * pr, 4 * g + 2 * pr + 1
                    pU = psU.tile([128, 512], F32, tag="pU")
                    for kc in range(4):
                        st, sp_ = (kc == 0), (kc == 3)
                        nc.tensor.matmul(pU[0:32, :], v_aug[:, kc, 33 * ha:33 * ha + 32],
                                         PT[:, kc, 2 * pr, :], start=st, stop=sp_,
                                         tile_position=(0, 0), skip_group_check=True)
                        nc.tensor.matmul(pU[32:64, :], v_aug[:, kc, 33 * hb:33 * hb + 32],
                                         PT[:, kc, 2 * pr + 1, :], start=st, stop=sp_,
                                         tile_position=(0, 32), skip_group_check=True)
                        nc.tensor.matmul(pU[64:96, :], ones_bf[:],
                                         PT[:, kc, 2 * pr, :], start=st, stop=sp_,
                                         tile_position=(0, 64), skip_group_check=True)
                        nc.tensor.matmul(pU[96:128, :], ones_bf[:],
                                         PT[:, kc, 2 * pr + 1, :], start=st, stop=sp_,
                                         tile_position=(0, 96), skip_group_check=True)
                    rz = sp.tile([128, 512], F32, tag="rz")
                    act_recip(rz[64:128, :], pU[64:128, :])
                    nc.vector.tensor_tensor(
                        out=aoT[64 * pr:64 * pr + 64, g, :],
                        in0=pU[0:64, :], in1=rz[64:128, :], op=OP.mult)

            if debug_seq is not None and s == debug_seq:
                nc.gpsimd.dma_start(out=dbg["dbg_qkT"][:], in_=qkT[:])
                dbg_va = sp.tile([128, 4, 264], F32, tag="dbg_va")
                nc.vector.tensor_copy(out=dbg_va[:], in_=v_aug[:])
                nc.sync.dma_start(out=dbg["dbg_vaug"][:], in_=dbg_va[:])
                dbg_ao = sp.tile([128, 2, 512], F32, tag="dbg_ao")
                nc.vector.tensor_copy(out=dbg_ao[:], in_=aoT[:])
                nc.sync.dma_start(out=dbg["dbg_aoT"][:], in_=dbg_ao[:])

            # out-projection + residual + LN1
            xpost = xp.tile([128, 4, 256], F32, tag=f"xpost{s % 2}")
            lnt = sp.tile([128, 4, 256], F32, tag="lnt")
            for mc in range(4):
                pr_ = psM.tile([128, 512], F32, tag="pm")
                for dc in range(2):
                    nc.tensor.matmul(pr_[:, 0:256], aoT[:, dc, mc * 128:(mc + 1) * 128],
                                     woutT[:, dc, :], start=(dc == 0), stop=(dc == 1))
                if not zero_bias:
                    nc.vector.tensor_tensor(out=pr_[:, 0:256], in0=pr_[:, 0:256],
                                            in1=boutf[:], op=OP.add)
                nc.vector.tensor_tensor(out=lnt[:, mc, :], in0=pr_[:, 0:256],
                                        in1=x_sb[:, mc, :], op=OP.add)
            layernorm4(lnt, g1f, be1f, xpost)

            return xpost, msk

        def layernorm4(t4, gf, bef, out4):
            """LN over last dim of t4 [128,4,256] -> out4 [128,4,256]."""
            mv4 = sp.tile([128, 4, 2], F32, tag="mv4")
            stats4 = sp.tile([128, 4, 6], F32, tag="stats4")
            for mc in range(4):
                nc.vector.bn_stats(out=stats4[:, mc, :], in_=t4[:, mc, :])
                nc.vector.bn_aggr(out=mv4[:, mc, :], in_=stats4[:, mc, :])
            nc.scalar.activation(out=mv4[:, :, 1], in_=mv4[:, :, 1],
                                 func=mybir.ActivationFunctionType.Sqrt, bias=epst[:])
            nc.vector.reciprocal(out=mv4[:, :, 1], in_=mv4[:, :, 1])
            for mc in range(4):
                if id_affine:
                    nc.vector.tensor_scalar(out=out4[:, mc, :], in0=t4[:, mc, :],
                                            scalar1=mv4[:, mc, 0:1], scalar2=mv4[:, mc, 1:2],
                                            op0=mybir.AluOpType.subtract,
                                            op1=mybir.AluOpType.mult)
                else:
                    nc.vector.tensor_scalar(out=t4[:, mc, :], in0=t4[:, mc, :],
                                            scalar1=mv4[:, mc, 0:1], scalar2=mv4[:, mc, 1:2],
                                            op0=mybir.AluOpType.subtract,
                                            op1=mybir.AluOpType.mult)
                    nc.vector.tensor_tensor(out=t4[:, mc, :], in0=t4[:, mc, :],
                                            in1=gf[:], op=mybir.AluOpType.mult)
                    nc.vector.tensor_tensor(out=out4[:, mc, :], in0=t4[:, mc, :],
                                            in1=bef[:], op=mybir.AluOpType.add)

        def ffn_ln2(s, xpost):
            xexT = sp.tile([128, 2, 512], BF, tag="xexT")
            for tc in range(4):
                pt_ = psM.tile([128, 512], F32, tag="pm")
                for dc in range(2):
                    nc.tensor.transpose(pt_[:, dc * 128:(dc + 1) * 128],
                                        xpost[:, tc, dc * 128:(dc + 1) * 128],
                                        idf32[:])
                nc.vector.tensor_copy(
                    out=xexT[:, :, tc * 128:(tc + 1) * 128],
                    in_=pt_[:, 0:256].rearrange("p (dc q) -> p dc q", dc=2))
            hT = sp.tile([128, 16, 512], BF, tag="hT")
            for f in range(16):
                ph = psM.tile([128, 512], F32, tag="pm")
                for dc in range(2):
                    nc.tensor.matmul(ph[:], w1T[:, dc, f * 128:(f + 1) * 128],
                                     xexT[:, dc, :], start=(dc == 0), stop=(dc == 1))
                if zero_bias:
                    if f % 2 == 0:
                        nc.vector.tensor_scalar_max(hT[:, f, :], ph[:], 0.0)
                    else:
                        nc.scalar.activation(out=hT[:, f, :], in_=ph[:],
                                             func=AF.Relu)
                else:
                    nc.vector.tensor_scalar(out=hT[:, f, :], in0=ph[:],
                                            scalar1=b1T[:, f:f + 1], scalar2=0.0,
                                            op0=mybir.AluOpType.add, op1=mybir.AluOpType.max)
            lnt2 = sp.tile([128, 4, 256], F32, tag="lnt2")
            of4 = sp.tile([128, 4, 256], F32, tag="of4")
            for tc in range(4):
                py = psM.tile([128, 512], F32, tag="pm")
                for f in range(16):
                    nc.tensor.matmul(py[:, 0:256], hT[:, f, tc * 128:(tc + 1) * 128],
                                     w2T[:, f, :], start=(f == 0), stop=(f == 15))
                if not zero_bias:
                    nc.vector.tensor_tensor(out=py[:, 0:256], in0=py[:, 0:256],
                                            in1=b2f[:], op=mybir.AluOpType.add)
                nc.vector.tensor_tensor(out=lnt2[:, tc, :], in0=py[:, 0:256],
                                        in1=xpost[:, tc, :], op=mybir.AluOpType.add)
            layernorm4(lnt2, g2f, be2f, of4)
            nc.scalar.dma_start(out=out_h[s].rearrange("c p d -> p c d"), in_=of4[:])

        for pair in range(n_pairs):
            xpostA, mskA = attention(2 * pair)
            xpostB, mskB = attention(2 * pair + 1)
            # means of post-LN1 (pre-exchange), replicated on all partitions
            means = []
            for st, xpost in ((0, xpostA), (1, xpostB)):
                pp = sp.tile([128, 256], F32, tag="pp")
                nc.vector.tensor_tensor(out=pp[:], in0=xpost[:, 0, :], in1=xpost[:, 1, :],
                                        op=mybir.AluOpType.add)
                nc.vector.tensor_tensor(out=pp[:], in0=pp[:], in1=xpost[:, 2, :],
                                        op=mybir.AluOpType.add)
                nc.vector.tensor_tensor(out=pp[:], in0=pp[:], in1=xpost[:, 3, :],
                                        op=mybir.AluOpType.add)
                pm = psM.tile([128, 512], F32, tag="pm")
                nc.tensor.matmul(pm[:, 0:256], onesf[:], pp[:], start=True, stop=True)
                mf = xp.tile([128, 256], F32, tag=f"mean{st}")
                nc.vector.tensor_scalar_mul(mf[:], pm[:, 0:256], 1.0 / 512.0)
                means.append(mf)
            if debug_seq is not None and 2 * pair == debug_seq:
                nc.sync.dma_start(out=dbg["dbg_mean"][0], in_=means[0][:])
                nc.sync.dma_start(out=dbg["dbg_mean"][1], in_=means[1][:])
            # exchange: replace masked tokens with other stream's mean
            for st, (xpost, msk) in enumerate(((xpostA, mskA), (xpostB, mskB))):
                other = means[1 - st]
                for tc in range(4):
                    nc.vector.copy_predicated(out=xpost[:, tc, :],
                                              mask=msk[:, tc:tc + 1].to_broadcast([128, 256]),
                                              data=other[:])
            if debug_seq is not None and 2 * pair == debug_seq:
                nc.sync.dma_start(out=dbg["dbg_xpost"][0], in_=xpostA[:])
                nc.sync.dma_start(out=dbg["dbg_xpost"][1], in_=xpostB[:])
            ffn_ln2(2 * pair, xpostA)
            ffn_ln2(2 * pair + 1, xpostB)

    nc.compile()
    return nc


# ---------------------------------------------------------------------------
# host packing
# ---------------------------------------------------------------------------

def pack_weights(W_in, b_in, W_out, b_out, W1, b1, W2, b2, g1, be1, g2, be2):
    def chunkp(a, c):  # [c*128, F] -> [128, c, F]
        return np.ascontiguousarray(a.reshape(c, 128, -1).transpose(1, 0, 2))

    wqk = W_in[:2 * D]                     # [512, 256]
    wv = W_in[2 * D:]                      # [256, 256]
    m = {
        "wqkT": chunkp(wqk.T, 2).astype(BF16),      # [128,2,512]
        "wvT": chunkp(wv.T, 2).astype(BF16),        # [128,2,256]
        "woutT": chunkp(W_out.T, 2).astype(BF16),   # [128,2,256]
        "w1T": chunkp(W1.T, 2).astype(BF16),        # [128,2,2048]
        "w2T": chunkp(W2.T, 16).astype(BF16),       # [128,16,256]
        "wqT32": chunkp(W_in[:D].T, 2).astype(np.float32),
        "wk32": chunkp(W_in[D:2 * D], 2).astype(np.float32),
        "bqk": np.ascontiguousarray(b_in[:2 * D].reshape(4, 128).T).astype(np.float32),
        "b1T": np.ascontiguousarray(b1.reshape(16, 128).T).astype(np.float32),
        "rows": np.stack([b_in[2 * D:], b_out, b2, g1, be1, g2, be2]).astype(np.float32),
    }
    return m


def pack_seqs(seqs):
    """seqs: [n, 512, 256] -> src [n,128,4,256], srcT [n,128,2,512]."""
    n = seqs.shape[0]
    src = np.ascontiguousarray(
        seqs.reshape(n, 4, 128, 256).transpose(0, 2, 1, 3)).astype(np.float32)
    st = seqs.transpose(0, 2, 1)  # [n, 256, 512]
    srcT = np.ascontiguousarray(
        st.reshape(n, 2, 128, 512).transpose(0, 2, 1, 3)).astype(np.float32)
    return src, srcT


_MODULE_CACHE = {}


def get_module(n_pairs=BPC, debug_seq=None, zero_bias=True, id_affine=True):
    key = (n_pairs, debug_seq, zero_bias, id_affine)
    if key not in _MODULE_CACHE:
        _MODULE_CACHE[key] = build_module(n_pairs, debug_seq, zero_bias, id_affine)
    return _MODULE_CACHE[key]


def kernel(src1, src2, AT_attn, AV_attn, W_in, b_in, W_out, b_out,
           W1, b1, W2, b2, g1, be1, g2, be2):
    from concourse.bass_utils import run_bass_kernel_spmd

    zb = all(float(np.abs(np.asarray(a)).max()) == 0.0
             for a in (b_in, b_out, b1, b2))
    ia = (float(np.abs(np.asarray(g1) - 1).max()) == 0 and float(np.abs(np.asarray(g2) - 1).max()) == 0
          and float(np.abs(np.asarray(be1)).max()) == 0 and float(np.abs(np.asarray(be2)).max()) == 0)
    nc = get_module(zero_bias=zb, id_affine=ia)
    wm = pack_weights(np.asarray(W_in, np.float32), np.asarray(b_in, np.float32),
                      np.asarray(W_out, np.float32), np.asarray(b_out, np.float32),
                      np.asarray(W1, np.float32), np.asarray(b1, np.float32),
                      np.asarray(W2, np.float32), np.asarray(b2, np.float32),
                      np.asarray(g1, np.float32), np.asarray(be1, np.float32),
                      np.asarray(g2, np.float32), np.asarray(be2, np.float32))
    src1 = np.asarray(src1, np.float32)
    src2 = np.asarray(src2, np.float32)
    in_maps = []
    for c in range(NC):
        bs = slice(c * BPC, (c + 1) * BPC)
        seqs = np.empty((2 * BPC, L, D), np.float32)
        seqs[0::2] = src1[bs]
        seqs[1::2] = src2[bs]
        src, srcT = pack_seqs(seqs)
        in_maps.append({"src": src, "srcT": srcT, **wm})
    res = run_bass_kernel_spmd(nc, in_maps, core_ids=list(range(NC)))
    x1 = np.empty((B, L, D), np.float32)
    x2 = np.empty((B, L, D), np.float32)
    for c in range(NC):
        o = res.results[c]["out"]  # [8, 4, 128, 256]
        full = o.reshape(2 * BPC, 512, 256)
        x1[c * BPC:(c + 1) * BPC] = full[0::2]
        x2[c * BPC:(c + 1) * BPC] = full[1::2]
    return np.stack([x1, x2])
